# revision 8
# baseline (speedup 1.0000x reference)
"""Trainium2 Bass kernel for nn_AttentionHead (B=512, T=256, C=384, H=64).

Strategy: pure data parallel over 8 NeuronCores (64 batches each).
Per batch on-chip:
  - x is pre-transposed on host to x^T layout [C, T] so the QKV projections
    can run with W chunks stationary and x^T moving (contraction C on
    partitions, 3 chunks of 128).
  - Q^T/K^T are produced in a single PSUM tile (Wq|Wk packed into one
    128-col stationary); V^T separately, then PE-transposed to V[T,H].
  - S = Q K^T with Q^T chunks stationary (2 chunks of Tq); the fully-masked
    (Tq=0, Tk=1) tile is skipped entirely.
  - softmax without max-subtraction (scores are O(10), exp is safe in f32);
    causal mask applied additively (-1e9) in PSUM; exp on ACT with fused
    row-sum (accum_out); 1/den folded into the output PSUM->SBUF copy.
  - P tiles are PE-transposed for the PV matmul (contraction Tk on
    partitions); out written as [Tq-chunk, 128, H] and re-flattened on host.
"""

import os
import sys

import numpy as np

for _p in ("/opt/trn_rl_repo", "/root/.axon_site/_ro/trn_rl_repo"):
    if os.path.isdir(_p) and _p not in sys.path:
        sys.path.append(_p)

from contextlib import ExitStack

import concourse.bass as bass
import concourse.mybir as mybir
import concourse.tile as tile
from concourse import bacc, bass_utils
from concourse.bass_utils import run_bass_kernel_spmd
from concourse.masks import make_causal_mask, make_identity


def _ensure_ntff_hook():
    """Register the axon NTFF profile hook if the image's antenv lacks it.

    The agent image's ``antenv`` package has no ``axon_hooks`` module, so
    boot() skips hook registration and ``trace=True`` would fail. Inject an
    in-memory module and register the ctypes-based hook from trn_agent_boot.
    """
    try:
        import types

        import antenv

        try:
            from antenv import axon_hooks  # noqa: F401

            return
        except ImportError:
            pass
        mod = types.ModuleType("antenv.axon_hooks")
        mod._hook = None
        mod.set_axon_ntff_profile_hook = lambda h: setattr(mod, "_hook", h)
        mod.get_axon_ntff_profile_hook = lambda: mod._hook
        sys.modules["antenv.axon_hooks"] = mod
        antenv.axon_hooks = mod

        from trn_agent_boot.trn_boot import _ntff_profile_via_ctypes

        so = os.environ.get("PJRT_LIBRARY_PATH", "/opt/axon/libaxon_pjrt.so")
        if os.path.exists(so):
            hook = _ntff_profile_via_ctypes(so)
            if hook is not None:
                mod._hook = hook
    except Exception:
        pass


_ensure_ntff_hook()
# artifact upload needs S3 access this container doesn't have
bass_utils.upload_artifacts = lambda tmpdir: str(tmpdir)

N_CORES = 8
B, T, C, H = 512, 256, 384, 64
BL = B // N_CORES          # 64 batches per core
NCH = C // 128             # 3 contraction chunks
SCALE = H ** -0.5
F32 = mybir.dt.float32
MASK_VAL = -1e9

LAST_EXEC_TIME_NS = None
_CACHED_NC = None


def _build():
    nc = bacc.Bacc()
    x_ext = nc.declare_dram_parameter("xt", [BL, 128, NCH, T], F32, isOutput=False)
    wqk_ext = nc.declare_dram_parameter("wqk", [128, NCH, 128], F32, isOutput=False)
    wv_ext = nc.declare_dram_parameter("wv", [128, NCH, H], F32, isOutput=False)
    bqk_ext = nc.declare_dram_parameter("bqk", [128, 1], F32, isOutput=False)
    bv_ext = nc.declare_dram_parameter("bv", [H, 1], F32, isOutput=False)
    out_ext = nc.declare_dram_parameter("out", [BL, 2, 128, H], F32, isOutput=True)

    with tile.TileContext(nc) as tc, ExitStack() as ctx:
        singles = ctx.enter_context(tc.tile_pool(name="singles", bufs=1))
        xp = ctx.enter_context(tc.tile_pool(name="xp", bufs=3))
        sb = ctx.enter_context(tc.tile_pool(name="sb", bufs=2))
        ob = ctx.enter_context(tc.tile_pool(name="ob", bufs=3))
        st = ctx.enter_context(tc.tile_pool(name="st", bufs=4))
        pp = ctx.enter_context(tc.tile_pool(name="pp", bufs=1, space="PSUM"))

        identity = singles.tile([128, 128], F32)
        make_identity(nc, identity)
        # mask[:, 0, :] = 0 (past block, unmasked), mask[:, 1, :] = causal block
        mask = singles.tile([128, 2, 128], F32)
        nc.gpsimd.memset(mask, 0.0)
        make_causal_mask(nc, mask[:, 1, :], mask_val=MASK_VAL)

        wqk = singles.tile([128, NCH, 128], F32)
        nc.sync.dma_start(out=wqk, in_=wqk_ext[:])
        wv = singles.tile([128, NCH, H], F32)
        nc.sync.dma_start(out=wv, in_=wv_ext[:])
        bqk = singles.tile([128, 1], F32)
        nc.sync.dma_start(out=bqk, in_=bqk_ext[:])
        bv = singles.tile([H, 1], F32)
        nc.sync.dma_start(out=bv, in_=bv_ext[:])

        for b in range(BL):
            xt = xp.tile([128, NCH, T], F32)
            nc.sync.dma_start(out=xt, in_=x_ext[b])

            # ---- projections: qk_ps rows 0:64 = Q^T (pre-scaled), 64:128 = K^T
            qk_ps = pp.tile([128, T], F32)
            for c in range(NCH):
                nc.tensor.matmul(
                    qk_ps, wqk[:, c, :], xt[:, c, :],
                    start=(c == 0), stop=(c == NCH - 1),
                )
            v_ps = pp.tile([H, T], F32)
            for c in range(NCH):
                nc.tensor.matmul(
                    v_ps, wv[:, c, :], xt[:, c, :],
                    start=(c == 0), stop=(c == NCH - 1),
                )

            # split the packed PSUM rows into separate base-0 SBUF tiles
            # (matmul operands must share a base partition); the 64-lane DVE
            # write can retarget quadrant 0 from a base-64 read window
            q_sb = sb.tile([H, T], F32)
            nc.vector.tensor_scalar_add(q_sb, qk_ps[:H, :], bqk[:H, :])
            k_sb = sb.tile([H, T], F32)
            nc.vector.tensor_scalar_add(k_sb, qk_ps[H:128, :], bqk[H:128, :])
            v_sb = sb.tile([H, T], F32)
            nc.vector.tensor_scalar_add(v_sb, v_ps, bv)

            # ---- V^T -> V via PE transpose: vt_sb[:, k, :] = V[k*128:(k+1)*128, :]
            vt_ps = pp.tile([128, 2, H], F32)
            for k in range(2):
                nc.tensor.transpose(
                    vt_ps[:, k, :], v_sb[:, k * 128:(k + 1) * 128], identity[:H, :H]
                )
            vt_sb = sb.tile([128, 2, H], F32)
            nc.vector.tensor_copy(vt_sb, vt_ps)

            # ---- scores (Tq chunk 0 only needs Tk 0:128)
            s0_ps = pp.tile([128, 128], F32)
            nc.tensor.matmul(
                s0_ps, q_sb[:, 0:128], k_sb[:, 0:128], start=True, stop=True
            )
            s1_ps = pp.tile([128, T], F32)
            nc.tensor.matmul(
                s1_ps, q_sb[:, 128:256], k_sb[:, :], start=True, stop=True
            )

            # ---- causal mask (additive, in place in PSUM)
            nc.vector.tensor_add(s0_ps, s0_ps, mask[:, 1, :])
            nc.vector.tensor_add(s1_ps, s1_ps, mask[:, :, :])

            # ---- exp with fused row-sum (no max subtraction; scores are small)
            p0_sb = sb.tile([128, 128], F32)
            den0 = st.tile([128, 1], F32)
            nc.scalar.activation(
                p0_sb, s0_ps, mybir.ActivationFunctionType.Exp, accum_out=den0
            )
            p1_sb = sb.tile([128, T], F32)
            den1 = st.tile([128, 1], F32)
            nc.scalar.activation(
                p1_sb, s1_ps, mybir.ActivationFunctionType.Exp, accum_out=den1
            )
            r0 = st.tile([128, 1], F32)
            nc.vector.reciprocal(r0, den0)
            r1 = st.tile([128, 1], F32)
            nc.vector.reciprocal(r1, den1)

            # ---- P^T via PE transpose: pt[:,0]=P^T[k0,q0], pt[:,1]=P^T[k0,q1],
            # pt[:,2]=P^T[k1,q1]
            pt_ps = pp.tile([128, 3, 128], F32)
            nc.tensor.transpose(pt_ps[:, 0, :], p0_sb, identity)
            nc.tensor.transpose(pt_ps[:, 1, :], p1_sb[:, 0:128], identity)
            nc.tensor.transpose(pt_ps[:, 2, :], p1_sb[:, 128:256], identity)
            pt_sb = sb.tile([128, 3, 128], F32)
            nc.vector.tensor_copy(pt_sb, pt_ps)

            # ---- out = P @ V, accumulated over Tk chunks
            o_ps = pp.tile([128, 2, H], F32)
            nc.tensor.matmul(
                o_ps[:, 0, :], pt_sb[:, 0, :], vt_sb[:, 0, :], start=True, stop=True
            )
            nc.tensor.matmul(
                o_ps[:, 1, :], pt_sb[:, 1, :], vt_sb[:, 0, :], start=True, stop=False
            )
            nc.tensor.matmul(
                o_ps[:, 1, :], pt_sb[:, 2, :], vt_sb[:, 1, :], start=False, stop=True
            )

            # ---- scale rows by 1/den on the way out of PSUM
            o_sb = ob.tile([128, 2, H], F32)
            nc.vector.tensor_scalar_mul(o_sb[:, 0, :], o_ps[:, 0, :], r0)
            nc.vector.tensor_scalar_mul(o_sb[:, 1, :], o_ps[:, 1, :], r1)

            nc.sync.dma_start(out=out_ext[b].rearrange("q p h -> p q h"), in_=o_sb)

    nc.compile()
    return nc


def _get_nc():
    global _CACHED_NC
    if _CACHED_NC is None:
        _CACHED_NC = _build()
    return _CACHED_NC


def kernel(x, Wq, bq, Wk, bk, Wv, bv):
    global LAST_EXEC_TIME_NS
    x = np.ascontiguousarray(np.asarray(x, dtype=np.float32))
    Wq = np.asarray(Wq, dtype=np.float32)
    bq = np.asarray(bq, dtype=np.float32)
    Wk = np.asarray(Wk, dtype=np.float32)
    bk = np.asarray(bk, dtype=np.float32)
    Wv = np.asarray(Wv, dtype=np.float32)
    bv = np.asarray(bv, dtype=np.float32)

    # fold the 1/sqrt(H) score scale into Wq/bq; pack [Wq|Wk] into one
    # 128-col stationary operand
    wqk = np.concatenate([Wq * SCALE, Wk], axis=1)           # [C, 128]
    wqk = np.ascontiguousarray(wqk.reshape(NCH, 128, 128).transpose(1, 0, 2))
    wv_h = np.ascontiguousarray(Wv.reshape(NCH, 128, H).transpose(1, 0, 2))
    bqk = np.ascontiguousarray(
        np.concatenate([bq * SCALE, bk]).reshape(128, 1).astype(np.float32)
    )
    bv_h = np.ascontiguousarray(bv.reshape(H, 1))

    in_maps = []
    for i in range(N_CORES):
        xs = x[i * BL:(i + 1) * BL]                          # [BL, T, C]
        xt = xs.transpose(0, 2, 1).reshape(BL, NCH, 128, T).transpose(0, 2, 1, 3)
        in_maps.append(
            {
                "xt": np.ascontiguousarray(xt),
                "wqk": wqk,
                "wv": wv_h,
                "bqk": bqk,
                "bv": bv_h,
            }
        )

    trace = os.environ.get("BASS_KERNEL_TRACE", "1") != "0"
    try:
        res = run_bass_kernel_spmd(
            _get_nc(), in_maps, core_ids=list(range(N_CORES)), trace=trace
        )
    except Exception:
        if not trace:
            raise
        res = run_bass_kernel_spmd(
            _get_nc(), in_maps, core_ids=list(range(N_CORES)), trace=False
        )
    LAST_EXEC_TIME_NS = res.exec_time_ns

    outs = [res.results[i]["out"].reshape(BL, T, H) for i in range(N_CORES)]
    return np.concatenate(outs, axis=0)
